# revision 18
# baseline (speedup 1.0000x reference)
"""Multi-head self-attention (causal) Trainium2 Bass/Tile kernel, 8-way SPMD.

Sharding: data-parallel over batch (4) x tensor-parallel over heads (2 groups
of 8 heads).  Core c handles batch c//2, head-group c%2.  Each core computes
q/k/v projections for its 512 local features, causal attention for its 8
heads, and a partial o-projection (contraction over its 512 features of the
attention output) giving a full-shape [S, D] partial that the host sums per
batch pair.

All matmul operands are bf16 (fp32 PSUM accumulation); softmax runs without
max-subtraction (scores ~ N(0,1) after the 1/8 scale, no overflow risk), with
exp on the scalar engine and the row-sum folded into the AV matmul via a ones
column appended to V.  Host pre-transposes inputs so no on-chip transposes
are needed:
  qT[e,s]  = wqT.T @ xT        (lhsT=wqT[d,e], rhs=xT[d,s])
  scoresT[sk,sq] = kT.T @ qT   (lhsT=kT[dk,sk], rhs=qT[dk,sq], K=64)
  avT[dk+1,sq]   = vaug.T @ expT  (lhsT=vaug[sk,65], rhs=expT[sk,sq])
  y[s,e]   = outT.T @ woT      (lhsT=outT[d,s], rhs=woT[d,e])

Causal handling: the 4 diagonal key tiles of each 512-query group are
NARROWED — queries below the tile's diagonal are not computed (scores, exp
and AV all run on the [off:512] query sub-range, off=128r), which is
numerically identical to masking them to zero.  The remaining intra-tile
triangle uses a single j>=i 0/1 mask (the pattern is independent of r).

Scheduling: projection / o-proj work is split into ~8-matmul units pulled
from a deque between attention kt steps, soaking up PE idle while the scalar
engine computes exp.  Scores for heads A/B go to separate 1-bank psum tiles
(PE row-disjoint tile pairs run concurrently); filler chains use a dedicated
psum tag so their PSUM->SBUF cast latency never stalls the score pipeline.
The kernel head keeps the PE spinning through the input DMA window (HAM
clock ramp) with DMAs spread over 4 queues.
"""

from collections import deque
from contextlib import ExitStack

import numpy as np
import ml_dtypes

import concourse.bass as bass
import concourse.tile as tile
from concourse import bacc, mybir
from concourse._compat import with_exitstack
from concourse.bass_utils import run_bass_kernel_spmd

B, S, D, H = 4, 2048, 1024, 16
DK = D // H          # 64
E = 512              # local features per core (8 heads)
HL = 8               # local heads
NCORES = 8
NDT = D // 128       # 8 d-tiles
NET = E // 128       # 4 e-tiles
NST = S // 128       # 16 s-tiles
NQG = S // 512       # 4 query groups

F32 = mybir.dt.float32
BF16 = mybir.dt.bfloat16
bf16 = ml_dtypes.bfloat16

_compiled = None
last_results = None  # test harness introspection


@with_exitstack
def _mhsa_kernel(ctx: ExitStack, tc: tile.TileContext, y, xT, wqT, wkT, wvT,
                 woT, mask):
    nc = tc.nc

    consts = ctx.enter_context(tc.tile_pool(name="consts", bufs=1))
    ex_pool = ctx.enter_context(tc.tile_pool(name="ex", bufs=8))
    rec_pool = ctx.enter_context(tc.tile_pool(name="rec", bufs=2))
    y_pool = ctx.enter_context(tc.tile_pool(name="ysb", bufs=4))
    ps_pool = ctx.enter_context(tc.tile_pool(name="psmm", bufs=2, space="PSUM"))
    av_pool = ctx.enter_context(tc.tile_pool(name="psav", bufs=2, space="PSUM"))

    def ctile(shape, dt_, tg):
        return consts.tile(shape, dt_, tag=tg, name=tg)

    # ---- persistent SBUF tiles -------------------------------------------
    xT_t = [ctile([128, S], BF16, f"xT{i}") for i in range(NDT)]
    wqT_t = [ctile([128, E], BF16, f"wqT{i}") for i in range(NDT)]
    wkT_t = [ctile([128, E], BF16, f"wkT{i}") for i in range(NDT)]
    wvT_t = [ctile([128, E], BF16, f"wvT{i}") for i in range(NDT)]
    woT_t = [ctile([128, D], BF16, f"woT{i}") for i in range(NET)]
    qT_t = [ctile([128, S], BF16, f"qT{i}") for i in range(NET)]
    kT_t = [ctile([128, S], BF16, f"kT{i}") for i in range(NET)]
    vaug_t = [ctile([128, HL * (DK + 1)], BF16, f"vaug{i}") for i in range(NST)]
    outT_t = [ctile([128, S], BF16, f"outT{i}") for i in range(NET)]
    mask_t = ctile([128, 512], BF16, "mask")

    # ones2: selector for the final pair's reciprocal broadcast matmul —
    # bc[j, :] = recb2[0, :] for j<64 (head A) and recb2[32, :] for j>=64
    # (head B).  Rows 0/32 because the DVE can only write at 32-aligned
    # partitions; K padded to 64 (a K=33 matmul wedges the exec unit).
    ones2 = ctile([64, 128], BF16, "ones2")
    warm = ctile([128, 512], BF16, "warm")
    # memsets FIRST on the vector queue so the PE warm-up isn't stuck
    # behind DMA-trigger instructions
    nc.vector.memset(warm, 0.0)
    nc.vector.memset(ones2, 0.0)
    nc.vector.memset(ones2[0:1, 0:64], 1.0)
    nc.vector.memset(ones2[32:33, 64:128], 1.0)

    # PE warm-up: HAM starts throttled at 1.2 GHz and needs ~3.4us of
    # sustained matmul activity to release; spin while the first DMAs land.
    def spin():
        wps = ps_pool.tile([128, 512], F32, tag="fil", name="wps")
        nc.tensor.matmul(wps, lhsT=warm[:, 0:128], rhs=warm,
                         start=True, stop=True)

    for _ in range(13):
        spin()

    # ---- input loads, spread over 4 queues so compute starts ASAP --------
    # proj_v(st0..3) needs wvT + xT cols 0:1024; qk(0,0) needs wq/wk.
    for i in range(6):
        nc.gpsimd.dma_start(out=xT_t[i][:, 0:1024],
                            in_=xT[i * 128:(i + 1) * 128, 0:1024])
    for i in range(NDT):
        nc.sync.dma_start(out=wqT_t[i], in_=wqT[i * 128:(i + 1) * 128, :])
        nc.scalar.dma_start(out=wkT_t[i], in_=wkT[i * 128:(i + 1) * 128, :])
    for i in (6, 7):
        nc.scalar.dma_start(out=xT_t[i][:, 0:1024],
                            in_=xT[i * 128:(i + 1) * 128, 0:1024])
    for i in range(4):
        nc.sync.dma_start(out=wvT_t[i], in_=wvT[i * 128:(i + 1) * 128, :])
        nc.scalar.dma_start(out=wvT_t[4 + i],
                            in_=wvT[(4 + i) * 128:(5 + i) * 128, :])
    for i in range(NDT):
        nc.gpsimd.dma_start(out=xT_t[i][:, 1024:2048],
                            in_=xT[i * 128:(i + 1) * 128, 1024:2048])
    nc.sync.dma_start(out=mask_t, in_=mask)
    for i in range(NET):
        nc.sync.dma_start(out=woT_t[i], in_=woT[i * 128:(i + 1) * 128, :])
    # dummy exp pulls the ~2.7us ACT_TABLE_LOAD off the first attention
    # slot; emitted after the scalar-queue DMA triggers so it doesn't
    # delay the weight loads.
    wex = ctile([1, 2], BF16, "wex")
    nc.scalar.activation(out=wex, in_=warm[0:1, 0:2],
                         func=mybir.ActivationFunctionType.Exp, scale=0.125)

    # Softmax denominators bounce through DRAM: DVE can only write at
    # 32-aligned base partitions, and SBUF APs cannot have a step-0
    # partition dim (needed for the broadcast) — DRAM APs can do both.
    sums_dram = nc.dram_tensor("sums_bounce", [NQG, HL, 512], F32).ap()
    rec_dram = nc.dram_tensor("rec_bounce", [NQG, HL, 512], BF16).ap()

    # ---- projection / o-proj units (~8 matmuls each) ----------------------
    def proj_qk_half(wt, dst, et, scg, hf):
        ps = ps_pool.tile([128, 512], F32, tag="fil", name="ps")
        s0 = scg * 1024 + hf * 512
        for dt_ in range(NDT):
            nc.tensor.matmul(
                ps,
                lhsT=wt[dt_][:, et * 128:(et + 1) * 128],
                rhs=xT_t[dt_][:, s0:s0 + 512],
                start=(dt_ == 0), stop=(dt_ == NDT - 1),
            )
        nc.vector.tensor_copy(dst[et][:, s0:s0 + 512], ps)

    def proj_v_st(st):
        # vaug tile [128, 8*65]: per-head 64 v columns + a ones column
        ps = ps_pool.tile([128, 512], F32, tag="fil", name="ps")
        for dt_ in range(NDT):
            nc.tensor.matmul(
                ps,
                lhsT=xT_t[dt_][:, st * 128:(st + 1) * 128],
                rhs=wvT_t[dt_],
                start=(dt_ == 0), stop=(dt_ == NDT - 1),
            )
        nc.vector.memset(vaug_t[st], 1.0)
        # one strided cast: [128, 8, 64] view skips the ones columns
        nc.vector.tensor_copy(
            vaug_t[st].rearrange("p (h c) -> p h c", c=65)[:, :, 0:64],
            ps.rearrange("p (h c) -> p h c", c=64),
        )

    def oproj_half(st, hf, q=0):
        ps = ps_pool.tile([128, 512], F32, tag="fil", name="ps")
        for dt_ in range(NET):
            nc.tensor.matmul(
                ps,
                lhsT=outT_t[dt_][:, st * 128:(st + 1) * 128],
                rhs=woT_t[dt_][:, hf * 512:(hf + 1) * 512],
                start=(dt_ == 0), stop=(dt_ == NET - 1),
            )
        ysb = y_pool.tile([128, 512], BF16, tag="ysb", name="ysb")
        nc.vector.tensor_copy(ysb, ps)
        eng = nc.gpsimd if q == 0 else nc.sync
        eng.dma_start(out=y[st * 128:(st + 1) * 128, hf * 512:(hf + 1) * 512],
                      in_=ysb)

    # ---- filler unit scheduling ------------------------------------------
    units = deque()
    done = {}
    need = {}

    def push(group, *fns):
        need[group] = need.get(group, 0) + len(fns)
        for f in fns:
            units.append((group, f))

    def run_one():
        if units:
            g, f = units.popleft()
            f()
            done[g] = done.get(g, 0) + 1
            return True
        return False

    def drain(group):
        while done.get(group, 0) < need.get(group, 0):
            if not run_one():
                raise RuntimeError(f"filler underflow for {group}")

    def qk_units(et, scg):
        g = f"qk{et}{scg}"
        return [(g, (lambda h=h, w=w, d=d: proj_qk_half(w, d, et, scg, h)))
                for w, d in ((wqT_t, qT_t), (wkT_t, kT_t)) for h in range(2)]

    def v_unit(st):
        return (f"v{st}", lambda st=st: proj_v_st(st))

    def o_units(st):
        return [(f"o{st}", lambda h=h, st=st: oproj_half(st, h))
                for h in range(2)]

    # ---- attention for one (head-pair, query-group) ----------------------
    # Heads hA=2*hp (partitions 0:64) and hB=2*hp+1 (64:128): per key tile
    # kt the two K=64 score matmuls go to separate 1-bank psum tiles and
    # disjoint PE row groups (tile positions (0,0)/(64,0)) so they run
    # concurrently.  outT stays UNNORMALIZED; denominators (the vaug ones
    # column, av row 64) are collected and normalization is batched.
    def attn(hp, qg):
        ti = hp
        hA, hB = 2 * hp, 2 * hp + 1
        nk = 4 * qg + 4
        avA = av_pool.tile([65, 512], F32, tag="av", name="avA")
        avB = av_pool.tile([65, 512], F32, tag="av", name="avB")

        def emit_av(kt, ex, off):
            narrow = off > 0
            for av, h in ((avA, hA), (avB, hB)):
                nc.tensor.matmul(
                    av[:, off:512],
                    lhsT=vaug_t[kt][:, h * 65:h * 65 + 65],
                    rhs=ex[:, (h & 1) * 512 + off:((h & 1) + 1) * 512],
                    start=(kt == 0), stop=(kt == nk - 1),
                    skip_group_check=narrow or kt == nk - 1,
                )

        pending = []
        for kt in range(nk):
            r = kt - 4 * qg
            off = 128 * r if r > 0 else 0
            ps = ps_pool.tile([128, 1024], F32, tag="s2", name="ps")
            for po in (0, 64):
                nc.tensor.matmul(
                    ps[:, po * 8 + off:po * 8 + 512],
                    lhsT=kT_t[ti][po:po + 64, kt * 128:(kt + 1) * 128],
                    rhs=qT_t[ti][po:po + 64, qg * 512 + off:(qg + 1) * 512],
                    start=True, stop=True,
                )
            ex = ex_pool.tile([128, 1024], BF16, tag="ex", name="ex")
            if off:
                for po in (0, 512):
                    nc.scalar.activation(
                        out=ex[:, po + off:po + 512],
                        in_=ps[:, po + off:po + 512],
                        func=mybir.ActivationFunctionType.Exp, scale=0.125)
            else:
                nc.scalar.activation(out=ex, in_=ps,
                                     func=mybir.ActivationFunctionType.Exp,
                                     scale=0.125)
            if r >= 0:  # diagonal strip: causal 0/1 mask on both heads —
                # ex column po+c holds query c, keep key i iff c-off >= i,
                # so the j>=i mask tile aligns at column 0, not at `off`.
                for po in (0, 512):
                    nc.vector.tensor_mul(ex[:, po + off:po + 512],
                                         ex[:, po + off:po + 512],
                                         mask_t[:, 0:512 - off])
            pending.append((kt, ex, off))
            if len(pending) > 2:  # lag 2: AV never waits on a fresh exp
                emit_av(*pending.pop(0))
            # soak exp latency with projection/o-proj filler work; the
            # attention steady state is ACT(exp)-bound at ~1.7x the PE's
            # attention work, so the pull rate ramps with qg: early
            # (filler-rich, PE-bound) groups pull sparsely, late groups
            # pull every kt.  When the deque runs dry late in the kernel a
            # warm-up spin keeps the PE ticking so HAM doesn't re-throttle
            # the clock for the tail.
            m = (2, 4, 3, 1)[qg]
            if kt % m == m - 1:
                if not run_one() and qg == 3:
                    spin()

        def flush_av():
            emit_av(*pending.pop(0))
            run_one()
            emit_av(*pending.pop(0))

        return flush_av, (lambda: _stash(hp, qg, ti, avA, avB))

    def _stash(hp, qg, ti, avA, avB):
        # stash unnormalized outputs + denominators; release av quickly
        hA, hB = 2 * hp, 2 * hp + 1
        if qg == NQG - 1 and hp == HL // 2 - 1:
            # final pair: no attention left to hide the DRAM-bounce latency
            # behind, so normalize inline via reciprocal + PE broadcast
            stg2 = rec_pool.tile([64, 512], F32, tag="stg2", name="stg2")
            nc.vector.memset(stg2, 1.0)
            for av, po, row in ((avA, 0, 0), (avB, 64, 32)):
                nc.vector.tensor_copy(
                    outT_t[ti][po:po + 64, qg * 512:(qg + 1) * 512],
                    av[0:64, :])
                nc.vector.tensor_copy(stg2[row:row + 1, :], av[64:65, :])
            rec2 = rec_pool.tile([64, 512], F32, tag="rec2", name="rec2")
            nc.vector.reciprocal_approx_fast(out=rec2, in_=stg2)
            recb2 = rec_pool.tile([64, 512], BF16, tag="recb2", name="recb2")
            nc.vector.tensor_copy(recb2, rec2)
            bc = av_pool.tile([128, 512], F32, tag="av", name="bc")
            nc.tensor.matmul(bc, lhsT=ones2, rhs=recb2, start=True, stop=True)
            for po in (0, 64):
                sl = outT_t[ti][po:po + 64, qg * 512:(qg + 1) * 512]
                nc.vector.tensor_mul(sl, sl, bc[po:po + 64, :])
        else:
            for av, h, po in ((avA, hA, 0), (avB, hB, 64)):
                nc.vector.tensor_copy(
                    outT_t[ti][po:po + 64, qg * 512:(qg + 1) * 512],
                    av[0:64, :])
                stg = rec_pool.tile([1, 512], F32, tag="stg", name="stg",
                                    bufs=4)
                nc.vector.tensor_copy(stg, av[64:65, :])
                nc.sync.dma_start(out=sums_dram[qg, h], in_=stg)

    # ---- batched normalization (DRAM-bounce broadcast) -------------------
    def _norm_heads(qg, heads):
        h0, nh = heads[0], len(heads)
        sums = rec_pool.tile([nh, 512], F32, tag=f"sums{nh}", name="sums")
        nc.sync.dma_start(out=sums, in_=sums_dram[qg, h0:h0 + nh])
        rec = rec_pool.tile([nh, 512], F32, tag=f"rec{nh}", name="rec")
        nc.vector.reciprocal_approx_fast(out=rec, in_=sums)
        recb = rec_pool.tile([nh, 512], BF16, tag=f"recb{nh}", name="recb")
        nc.vector.tensor_copy(recb, rec)
        nc.sync.dma_start(out=rec_dram[qg, h0:h0 + nh], in_=recb)
        for h in heads:
            ti, po = h // 2, 64 * (h % 2)
            # walrus requires SBUF tensor_tensor inputs to share the start
            # partition, so land the broadcast at the same partition range
            bcs = rec_pool.tile([128, 512], BF16, tag="bcs", name="bcs")
            nc.sync.dma_start(
                out=bcs[po:po + 64, :],
                in_=rec_dram[qg, h:h + 1, :].to_broadcast([64, 512]))
            sl = outT_t[ti][po:po + 64, qg * 512:(qg + 1) * 512]
            nc.vector.tensor_mul(sl, sl, bcs[po:po + 64, :])

    def normalize_pair(qg, hp):
        _norm_heads(qg, [2 * hp, 2 * hp + 1])

    def n_units(qg):
        return [(f"n{qg}", lambda hp=hp: normalize_pair(qg, hp))
                for hp in range(4)]

    # ---- program order ----------------------------------------------------
    # Immediate prerequisites of the first attention slot; qk first (its
    # weight DMAs land first), proj_v after (wvT arrives later).
    for hf in range(2):
        proj_qk_half(wqT_t, qT_t, 0, 0, hf)
        proj_qk_half(wkT_t, kT_t, 0, 0, hf)
    for st in range(4):
        proj_v_st(st)

    # NOTE on ordering: normalize units of query-group qg are pushed only
    # after slot (qg, 3)'s stash (sums_bounce RAW is ordered by sync-queue
    # program order), and oproj units of qg only after normalize(qg) is in
    # the queue (FIFO then guarantees run order).  o10/o11 are reserved out
    # of the deque: they run between the final slot's AV flush and its
    # inline normalization so the PE chews them during the reciprocal chain.
    push_before = {
        (0, 0): qk_units(1, 0),
        (0, 1): qk_units(2, 0) + [v_unit(4), v_unit(5)],
        (0, 2): qk_units(3, 0) + [v_unit(6), v_unit(7)],
        (1, 0): qk_units(0, 1) + [v_unit(8), v_unit(9)],
        (1, 1): qk_units(1, 1) + [v_unit(10), v_unit(11)],
        (1, 2): qk_units(2, 1) + [v_unit(12), v_unit(13)],
        (1, 3): qk_units(3, 1) + [v_unit(14), v_unit(15)],
        (2, 0): o_units(0) + o_units(1),
        (2, 1): o_units(2) + o_units(3),
        (2, 2): o_units(4) + o_units(5),
        (2, 3): o_units(6) + o_units(7),
        (3, 0): o_units(8),
        (3, 1): o_units(9),
    }
    push_after = {
        (0, 3): n_units(0),
        (1, 3): n_units(1),
        (2, 3): n_units(2),
    }
    for qg in range(NQG):
        for hp in range(HL // 2):
            for g, f in push_before.get((qg, hp), []):
                push(g, f)
            drain(f"qk{hp}0")
            if qg >= 2:
                drain(f"qk{hp}1")
            for st in range(4 * qg + 4):
                drain(f"v{st}")
            flush_av, stash = attn(hp, qg)
            if (qg, hp) == (3, 3):
                # reserved PE work emitted BEFORE the tail AVs (which wait
                # on the last exps) so the PE chews it instead of stalling;
                # it also covers the inline-normalize reciprocal chain.
                for st in (10, 11):
                    for hf in range(2):
                        oproj_half(st, hf)
            flush_av()
            stash()
            if qg == 3 and hp < 3:
                normalize_pair(3, hp)
            for g, f in push_after.get((qg, hp), []):
                push(g, f)
    while run_one():
        pass
    for st in range(4 * (NQG - 1), 4 * NQG):
        for hf in range(2):
            oproj_half(st, hf, q=hf)


def _build():
    nc = bacc.Bacc("TRN2", target_bir_lowering=False, debug=False,
                   num_devices=NCORES)
    xT = nc.dram_tensor("xT", [D, S], BF16, kind="ExternalInput").ap()
    wqT = nc.dram_tensor("wqT", [D, E], BF16, kind="ExternalInput").ap()
    wkT = nc.dram_tensor("wkT", [D, E], BF16, kind="ExternalInput").ap()
    wvT = nc.dram_tensor("wvT", [D, E], BF16, kind="ExternalInput").ap()
    woT = nc.dram_tensor("woT", [E, D], BF16, kind="ExternalInput").ap()
    mask = nc.dram_tensor("mask", [128, 512], BF16,
                          kind="ExternalInput").ap()
    y = nc.dram_tensor("y", [S, D], BF16, kind="ExternalOutput").ap()
    with tile.TileContext(nc) as tc:
        _mhsa_kernel(tc, y, xT, wqT, wkT, wvT, woT, mask)
    nc.compile()
    return nc


def get_compiled():
    global _compiled
    if _compiled is None:
        _compiled = _build()
    return _compiled


def _make_masks():
    # j >= i keep-mask for the in-tile triangle of every diagonal key tile:
    # after narrowing to the [off:512] query sub-range (off = 128*r), key
    # 128*r+i is kept for packed query column j iff j >= i, independent of r.
    m = (np.arange(512)[None, :] >= np.arange(128)[:, None])
    return m.astype(bf16)


def kernel(**inputs):
    global last_results
    x = np.asarray(inputs["in_features"], dtype=np.float32)
    w_q = np.asarray(inputs["w_q"], dtype=np.float32)
    w_k = np.asarray(inputs["w_k"], dtype=np.float32)
    w_v = np.asarray(inputs["w_v"], dtype=np.float32)
    w_o = np.asarray(inputs["w_o"], dtype=np.float32)

    nc = get_compiled()
    mask = _make_masks()
    in_maps = []
    for c in range(NCORES):
        b, hg = divmod(c, 2)
        es = slice(hg * E, (hg + 1) * E)
        in_maps.append({
            "xT": x[b].T.astype(bf16),
            "wqT": w_q[es, :].T.astype(bf16),
            "wkT": w_k[es, :].T.astype(bf16),
            "wvT": w_v[es, :].T.astype(bf16),
            "woT": w_o[:, es].T.astype(bf16),
            "mask": mask,
        })
    res = run_bass_kernel_spmd(nc, in_maps, list(range(NCORES)))
    last_results = res
    y = np.zeros((B, S, D), dtype=np.float32)
    for c in range(NCORES):
        y[c // 2] += res.results[c]["y"].astype(np.float32)
    return y
